# revision 12
# baseline (speedup 1.0000x reference)
"""Trainium2 Bass kernel for nn_DAGDecoder (dense_mlp).

Reference computation (B=1024, L=256, H=512, N=128 nodes, T=16 types):
    h  = relu(z @ W_z + b_z)                     [B, H]
    node_logits[b, n, :] = h[b] @ W_node + b_node      (same for all n)
    p  = sigmoid(relu(cat(h, h) @ W_e1 + b_e1) @ W_e2 + b_e2)   [B, 1]
    edge_probs[b] = p[b] * tril(ones(N, N), -1)  [B, N, N]

Host-side algebraic folds:
    cat(h, h) @ W_e1 == h @ (W_e1[:H] + W_e1[H:])  -> one [H, H] matmul
    node_logits is a broadcast of a [B, T] matrix over the node axis.
    edge_probs is a rank-1 outer product p x tril_flat per batch row.

Sharding: pure data parallel over batch; 8 cores x 128 batch rows; the
batch shard exactly fills the 128 SBUF partitions. Weights replicated.

On-device layout is fully transposed (hT [H, Bs], e1T [H, Bs], pT [1, Bs])
so every matmul contraction dim sits on partitions. Precision plan:
  - h and node_logits path: fp32 matmuls (node output ~1e-7 rel)
  - e1/p path: bf16 matmuls with fp32 PSUM accumulate (feeds only the
    scalar p through a sigmoid; ~5e-4 rel)
  - edge outer product: near-exact via bf16 hi/lo split of p
    (p == p_hi + p_lo, each bf16; the tril mask is exact in bf16), K=2
    bf16 matmuls instead of the 4-pass fp32 LOW_HIGH path.
Biases ride either the scalar-engine activation (per-partition bias) or a
K=1 matmul accumulation (ones-row x bias-row) when the bias is along the
free axis.
"""

import numpy as np

import concourse.bass as bass
import concourse.mybir as mybir
import concourse.tile as tile
from concourse.bass_utils import run_bass_kernel_spmd
from concourse.vector_clock import ScopedClock

# ---------------------------------------------------------------------------
# Workaround: the walrus build in this container encodes at most ONE sync
# wait command per instruction ("Too many sync wait commands"). Tile emits
# multi-wait instructions routinely. Engines execute their stream in order,
# so hoisting extra waits onto same-engine NOPs immediately before the
# instruction is semantically identical.
_MAX_DRAIN_WAITS = 1


def _drain_and_barrier_split(self, tick_clock, wait_clock):
    drain_inst = self.nc.sync.drain()
    wait_clock.add_sem_waits(
        drain_inst.ins, ScopedClock({None: tick_clock.global_clock})
    )
    waits = list(drain_inst.ins.sync_info.on_wait)
    if len(waits) > _MAX_DRAIN_WAITS:
        drain_inst.ins.sync_info = mybir.SyncInfo(
            on_wait=waits[:_MAX_DRAIN_WAITS], on_update=[]
        )
        for i in range(_MAX_DRAIN_WAITS, len(waits), _MAX_DRAIN_WAITS):
            d2 = self.nc.sync.drain()
            d2.ins.sync_info = mybir.SyncInfo(
                on_wait=waits[i : i + _MAX_DRAIN_WAITS], on_update=[]
            )

    self.nc.all_engine_barrier()
    assert self.sems is not None
    popped = self.nc._tile_sem_poison_stack.pop()
    assert popped is self._sem_poison
    self.nc.clear_and_free_semaphores(list(self.sems.allocated().values()))
    self.nc.all_engine_barrier()


tile.TileContext._drain_and_barrier = _drain_and_barrier_split


def _split_multi_waits(nc):
    for f in nc.m.functions:
        for bb in f.blocks:
            old = list(bb.instructions)
            new = []
            changed = False
            for ins in old:
                si = ins.sync_info
                waits = list(si.on_wait) if si is not None else []
                if len(waits) > 1:
                    changed = True
                    for i, w in enumerate(waits[:-1]):
                        new.append(
                            mybir.InstNoOp(
                                name=f"{ins.name}_wsplit{i}",
                                engine=ins.engine,
                                bass_nofuse=True,
                                sync_info=mybir.SyncInfo(on_wait=[w], on_update=[]),
                            )
                        )
                    ins.sync_info = mybir.SyncInfo(
                        on_wait=[waits[-1]], on_update=list(si.on_update)
                    )
                new.append(ins)
            if changed:
                bb.instructions = new


# ---------------------------------------------------------------------------

N_CORES = 8
B = 1024
BS = B // N_CORES  # 128 batch rows per core == SBUF partition count
L = 256  # latent
H = 512  # hidden
T = 16  # node types
NN = 128  # num_nodes

F32 = mybir.dt.float32
BF16 = mybir.dt.bfloat16
_NC_CACHE = {}
# Extra kwargs for run_bass_kernel_spmd (test harness sets trace=True here).
_RUN_KWARGS = {}


def _build_bass():
    nc = bass.Bass("TRN2", target_bir_lowering=False, debug=False)

    zt = nc.dram_tensor("zt", [L, BS], F32, kind="ExternalInput")
    wz = nc.dram_tensor("wz", [L, H], F32, kind="ExternalInput")
    we1 = nc.dram_tensor("we1", [H, H], BF16, kind="ExternalInput")
    wn = nc.dram_tensor("wn", [H, T], F32, kind="ExternalInput")
    cb = nc.dram_tensor("cb", [128, 12], F32, kind="ExternalInput")
    cbb = nc.dram_tensor("cbb", [128, 4], BF16, kind="ExternalInput")
    cr = nc.dram_tensor("cr", [1, 160], F32, kind="ExternalInput")
    trilf = nc.dram_tensor("trilf", [2, NN * NN], BF16, kind="ExternalInput")

    node_out = nc.dram_tensor("node_out", [BS, NN * T], F32, kind="ExternalOutput")
    edge_out = nc.dram_tensor("edge_out", [BS, NN * NN], F32, kind="ExternalOutput")

    RELU = mybir.ActivationFunctionType.Relu
    SIGM = mybir.ActivationFunctionType.Sigmoid

    with tile.TileContext(nc) as tc:
        with (
            tc.tile_pool(name="wts", bufs=1) as wts,
            tc.tile_pool(name="acts", bufs=1) as acts,
            tc.tile_pool(name="edge", bufs=4) as edge,
            tc.tile_pool(name="psA", bufs=2, space=bass.MemorySpace.PSUM) as psA,
            tc.tile_pool(name="psE", bufs=4, space=bass.MemorySpace.PSUM) as psE,
        ):
            # ---- loads (HWDGE rings; critical-path tensors on SP) ------
            zt_t = [wts.tile([128, BS], F32, name=f"zt{k}", tag=f"zt{k}") for k in range(2)]
            for k in range(2):
                nc.sync.dma_start(zt_t[k][:], zt.ap()[k * 128 : (k + 1) * 128, :])
            wz_t = [wts.tile([128, H], F32, name=f"wz{k}", tag=f"wz{k}") for k in range(2)]
            for k in range(2):
                nc.sync.dma_start(wz_t[k][:], wz.ap()[k * 128 : (k + 1) * 128, :])
            we1_t = [wts.tile([128, H], BF16, name=f"we1{k}", tag=f"we1{k}") for k in range(4)]
            for k in range(4):
                nc.scalar.dma_start(we1_t[k][:], we1.ap()[k * 128 : (k + 1) * 128, :])
            wn_t = [wts.tile([128, T], F32, name=f"wn{k}", tag=f"wn{k}") for k in range(4)]
            for k in range(4):
                nc.scalar.dma_start(wn_t[k][:], wn.ap()[k * 128 : (k + 1) * 128, :])
            cb_t = wts.tile([128, 12], F32, name="cb", tag="cb")
            nc.sync.dma_start(cb_t[:], cb.ap())
            cbb_t = wts.tile([128, 4], BF16, name="cbb", tag="cbb")
            nc.scalar.dma_start(cbb_t[:], cbb.ap())
            cr_t = wts.tile([1, 160], F32, name="cr", tag="cr")
            nc.sync.dma_start(cr_t[:], cr.ap())
            tril_t = wts.tile([2, NN * NN], BF16, name="tril", tag="tril")
            nc.scalar.dma_start(tril_t[:], trilf.ap())

            # ---- hT[m] = relu(W_z[:, m].T @ zT + b_z[m])  fp32 ---------
            ht = [acts.tile([128, BS], F32, name=f"ht{m}", tag=f"ht{m}") for m in range(4)]
            htb = [acts.tile([128, BS], BF16, name=f"htb{m}", tag=f"htb{m}") for m in range(4)]
            for m in range(4):
                ps = psA.tile([128, BS], F32)
                for k in range(2):
                    nc.tensor.matmul(
                        ps[:],
                        wz_t[k][:, m * 128 : (m + 1) * 128],
                        zt_t[k][:],
                        start=(k == 0),
                        stop=(k == 1),
                    )
                nc.scalar.activation(ht[m][:], ps[:], RELU, bias=cb_t[:, m : m + 1])
                nc.vector.tensor_copy(htb[m][:], ht[m][:])

            # ---- e1T[m] = relu(W_e1f[:, m].T @ hT + b_e1[m])  bf16 -----
            e1 = [acts.tile([128, BS], BF16, name=f"e1{m}", tag=f"e1{m}") for m in range(4)]
            for m in range(4):
                ps = psA.tile([128, BS], F32)
                for k in range(4):
                    nc.tensor.matmul(
                        ps[:],
                        we1_t[k][:, m * 128 : (m + 1) * 128],
                        htb[k][:],
                        start=(k == 0),
                        stop=(k == 3),
                    )
                nc.scalar.activation(
                    e1[m][:], ps[:], RELU, bias=cb_t[:, 4 + m : 5 + m]
                )

            # ---- pT = sigmoid(W_e2.T @ e1T + b_e2)  [1, BS] ------------
            psp = psA.tile([1, BS], F32, bufs=1)
            for k in range(4):
                nc.tensor.matmul(
                    psp[:],
                    cbb_t[:, k : k + 1],
                    e1[k][:],
                    start=(k == 0),
                    stop=(k == 3),
                )
            pt = acts.tile([1, BS], F32, name="pt", tag="pt")
            nc.scalar.activation(pt[:], psp[:], SIGM, bias=cr_t[0:1, 144:145])
            # exact fp32 -> bf16 hi/lo split: p == hi + lo to ~2^-17 rel.
            # Compute engines cannot address partition 1 directly, so build
            # hi/lo on partition-0 tiles and assemble [2, BS] via two tiny
            # SBUF->SBUF DMAs on separate rings.
            pthi_b = acts.tile([1, BS], BF16, name="pthi_b", tag="pthi_b")
            nc.vector.tensor_copy(pthi_b[:], pt[:])
            pthi_f = acts.tile([1, BS], F32, name="pthi_f", tag="pthi_f")
            nc.vector.tensor_copy(pthi_f[:], pthi_b[:])
            plo_f = acts.tile([1, BS], F32, name="plo_f", tag="plo_f")
            nc.vector.tensor_sub(plo_f[:], pt[:], pthi_f[:])
            plo_b = acts.tile([1, BS], BF16, name="plo_b", tag="plo_b")
            nc.vector.tensor_copy(plo_b[:], plo_f[:])
            pt2 = acts.tile([2, BS], BF16, name="pt2", tag="pt2")
            nc.sync.dma_start(pt2[0:1, :], pthi_b[:])
            nc.scalar.dma_start(pt2[1:2, :], plo_b[:])

            # ---- node logits: nl = hT.T @ W_node (+ ones x b_node) fp32
            psn = psA.tile([128, T], F32, bufs=1)
            for k in range(4):
                nc.tensor.matmul(
                    psn[:], ht[k][:], wn_t[k][:], start=(k == 0), stop=False
                )
            nc.tensor.matmul(
                psn[:], cr_t[0:1, 16:144], cr_t[0:1, 0:16], start=False, stop=True
            )
            nt = acts.tile([128, NN * T], F32, name="nt", tag="nt")
            nc.vector.tensor_copy(nt[:, 0:T], psn[:])
            w = T
            while w < NN * T:
                nc.vector.tensor_copy(nt[:, w : 2 * w], nt[:, 0:w])
                w *= 2
            nc.gpsimd.dma_start(node_out.ap(), nt[:])

            # ---- edge: out[b, f] = (p_hi + p_lo)[b] * tril_flat[f] -----
            # 8 macro-chunks of 2048; each = 4 K=2 bf16 matmuls
            # [2,128]x[2,512] -> PSUM f32 -> copy (ACT/DVE) -> 1MB DMA.
            for c in range(8):
                et = edge.tile([128, 2048], F32, name="et", tag="et")
                for j in range(4):
                    f0 = c * 2048 + j * 512
                    pse = psE.tile([128, 512], F32)
                    nc.tensor.matmul(
                        pse[:],
                        pt2[:],
                        tril_t[:, f0 : f0 + 512],
                        start=True,
                        stop=True,
                    )
                    dst = et[:, j * 512 : (j + 1) * 512]
                    if (c * 4 + j) % 2 == 0:
                        nc.scalar.copy(dst, pse[:])
                    else:
                        nc.vector.tensor_copy(dst, pse[:])
                if c % 2 == 0:
                    nc.sync.dma_start(
                        edge_out.ap()[:, c * 2048 : (c + 1) * 2048], et[:]
                    )
                else:
                    nc.scalar.dma_start(
                        edge_out.ap()[:, c * 2048 : (c + 1) * 2048], et[:]
                    )

    _split_multi_waits(nc)
    return nc


def kernel(z, num_nodes, W_z, b_z, W_node, b_node, W_e1, b_e1, W_e2, b_e2):
    import ml_dtypes

    z = np.asarray(z, dtype=np.float32)
    W_z = np.asarray(W_z, dtype=np.float32)
    b_z = np.asarray(b_z, dtype=np.float32)
    W_node = np.asarray(W_node, dtype=np.float32)
    b_node = np.asarray(b_node, dtype=np.float32)
    W_e1 = np.asarray(W_e1, dtype=np.float32)
    b_e1 = np.asarray(b_e1, dtype=np.float32)
    W_e2 = np.asarray(W_e2, dtype=np.float32)
    b_e2 = np.asarray(b_e2, dtype=np.float32)
    n_nodes = int(num_nodes)
    assert n_nodes == NN, f"kernel hardcodes num_nodes={NN}, got {n_nodes}"
    assert z.shape == (B, L)

    # Host-side constant packing (tiny tensors, replicated to all cores).
    we1f = np.ascontiguousarray(
        (W_e1[:H] + W_e1[H:]).astype(ml_dtypes.bfloat16)
    )  # [H, H] bf16
    cb = np.zeros((128, 12), dtype=np.float32)
    cb[:, 0:4] = b_z.reshape(4, 128).T
    cb[:, 4:8] = b_e1.reshape(4, 128).T
    cb[:, 8:12] = W_e2[:, 0].reshape(4, 128).T
    cbb = np.ascontiguousarray(
        W_e2[:, 0].reshape(4, 128).T.astype(ml_dtypes.bfloat16)
    )  # [128, 4] bf16
    cr = np.zeros((1, 160), dtype=np.float32)
    cr[0, 0:T] = b_node
    cr[0, 16:144] = 1.0
    cr[0, 144] = b_e2[0]
    mask = np.tril(np.ones((NN, NN), dtype=np.float32), -1).reshape(1, NN * NN)
    trilf = np.ascontiguousarray(np.tile(mask, (2, 1)).astype(ml_dtypes.bfloat16))

    if "nc" not in _NC_CACHE:
        _NC_CACHE["nc"] = _build_bass()
    nc = _NC_CACHE["nc"]

    shared = {
        "wz": np.ascontiguousarray(W_z),
        "we1": we1f,
        "wn": np.ascontiguousarray(W_node),
        "cb": cb,
        "cbb": cbb,
        "cr": cr,
        "trilf": trilf,
    }
    in_maps = []
    for i in range(N_CORES):
        m = dict(shared)
        m["zt"] = np.ascontiguousarray(z[i * BS : (i + 1) * BS].T)
        in_maps.append(m)

    res = run_bass_kernel_spmd(nc, in_maps, core_ids=list(range(N_CORES)), **_RUN_KWARGS)
    _NC_CACHE["last_result"] = res

    node_logits = np.concatenate(
        [res.results[i]["node_out"].reshape(BS, NN, T) for i in range(N_CORES)],
        axis=0,
    )
    edge_probs = np.concatenate(
        [res.results[i]["edge_out"].reshape(BS, NN, NN) for i in range(N_CORES)],
        axis=0,
    )
    return node_logits, edge_probs


# revision 15
# speedup vs baseline: 1.0633x; 1.0633x over previous
"""Trainium2 Bass kernel for nn_DAGDecoder (dense_mlp).

Reference computation (B=1024, L=256, H=512, N=128 nodes, T=16 types):
    h  = relu(z @ W_z + b_z)                     [B, H]
    node_logits[b, n, :] = h[b] @ W_node + b_node      (same for all n)
    p  = sigmoid(relu(cat(h, h) @ W_e1 + b_e1) @ W_e2 + b_e2)   [B, 1]
    edge_probs[b] = p[b] * tril(ones(N, N), -1)  [B, N, N]

Host-side algebraic folds:
    cat(h, h) @ W_e1 == h @ (W_e1[:H] + W_e1[H:])  -> one [H, H] matmul
    node_logits is a broadcast of a [B, T] matrix over the node axis.
    edge_probs is a rank-1 outer product p x tril_flat per batch row.

Sharding: pure data parallel over batch; 8 cores x 128 batch rows; the
batch shard exactly fills the 128 SBUF partitions. Weights replicated.

On-device layout is fully transposed (hT [H, Bs], e1T [H, Bs], pT [1, Bs])
so every matmul contraction dim sits on partitions. Precision plan:
  - h and node_logits path: fp32 matmuls (node output ~1e-7 rel)
  - e1/p path: bf16 matmuls with fp32 PSUM accumulate (feeds only the
    scalar p through a sigmoid; ~5e-4 rel)
  - edge outer product: near-exact via bf16 hi/lo split of p
    (p == p_hi + p_lo, each bf16; the tril mask is exact in bf16), K=2
    bf16 matmuls instead of the 4-pass fp32 LOW_HIGH path.
Biases ride either the scalar-engine activation (per-partition bias) or a
K=1 matmul accumulation (ones-row x bias-row) when the bias is along the
free axis.
"""

import numpy as np

import concourse.bass as bass
import concourse.mybir as mybir
import concourse.tile as tile
from concourse.bass_utils import run_bass_kernel_spmd
from concourse.vector_clock import ScopedClock

# ---------------------------------------------------------------------------
# Workaround: the walrus build in this container encodes at most ONE sync
# wait command per instruction ("Too many sync wait commands"). Tile emits
# multi-wait instructions routinely. Engines execute their stream in order,
# so hoisting extra waits onto same-engine NOPs immediately before the
# instruction is semantically identical.
_MAX_DRAIN_WAITS = 1


def _drain_and_barrier_split(self, tick_clock, wait_clock):
    drain_inst = self.nc.sync.drain()
    wait_clock.add_sem_waits(
        drain_inst.ins, ScopedClock({None: tick_clock.global_clock})
    )
    waits = list(drain_inst.ins.sync_info.on_wait)
    if len(waits) > _MAX_DRAIN_WAITS:
        drain_inst.ins.sync_info = mybir.SyncInfo(
            on_wait=waits[:_MAX_DRAIN_WAITS], on_update=[]
        )
        for i in range(_MAX_DRAIN_WAITS, len(waits), _MAX_DRAIN_WAITS):
            d2 = self.nc.sync.drain()
            d2.ins.sync_info = mybir.SyncInfo(
                on_wait=waits[i : i + _MAX_DRAIN_WAITS], on_update=[]
            )

    self.nc.all_engine_barrier()
    assert self.sems is not None
    popped = self.nc._tile_sem_poison_stack.pop()
    assert popped is self._sem_poison
    self.nc.clear_and_free_semaphores(list(self.sems.allocated().values()))


tile.TileContext._drain_and_barrier = _drain_and_barrier_split


def _split_multi_waits(nc):
    for f in nc.m.functions:
        for bb in f.blocks:
            old = list(bb.instructions)
            new = []
            changed = False
            for ins in old:
                si = ins.sync_info
                waits = list(si.on_wait) if si is not None else []
                if len(waits) > 1:
                    changed = True
                    for i, w in enumerate(waits[:-1]):
                        new.append(
                            mybir.InstNoOp(
                                name=f"{ins.name}_wsplit{i}",
                                engine=ins.engine,
                                bass_nofuse=True,
                                sync_info=mybir.SyncInfo(on_wait=[w], on_update=[]),
                            )
                        )
                    ins.sync_info = mybir.SyncInfo(
                        on_wait=[waits[-1]], on_update=list(si.on_update)
                    )
                new.append(ins)
            if changed:
                bb.instructions = new


# ---------------------------------------------------------------------------

N_CORES = 8
B = 1024
BS = B // N_CORES  # 128 batch rows per core == SBUF partition count
L = 256  # latent
H = 512  # hidden
T = 16  # node types
NN = 128  # num_nodes

F32 = mybir.dt.float32
BF16 = mybir.dt.bfloat16
_NC_CACHE = {}
# Extra kwargs for run_bass_kernel_spmd (test harness sets trace=True here).
_RUN_KWARGS = {}


def _build_bass():
    nc = bass.Bass("TRN2", target_bir_lowering=False, debug=False)

    # bigA: [zt0 | wz0 | zt1 | wz1] columns (f32); bigB: [we1 k-tiles | cbb]
    # (bf16); bigC: [cb | wn k-tiles] (f32). Packing makes each load ONE
    # large DMA instead of many sequencer-serialized issues.
    bigA = nc.dram_tensor("bigA", [128, 1280], F32, kind="ExternalInput")
    bigB = nc.dram_tensor("bigB", [128, 2052], BF16, kind="ExternalInput")
    bigC = nc.dram_tensor("bigC", [128, 76], F32, kind="ExternalInput")
    cr = nc.dram_tensor("cr", [1, 160], F32, kind="ExternalInput")
    trilf = nc.dram_tensor("trilf", [2, NN * NN], BF16, kind="ExternalInput")

    node_out = nc.dram_tensor("node_out", [BS, NN * T], F32, kind="ExternalOutput")
    edge_out = nc.dram_tensor("edge_out", [BS, NN * NN], F32, kind="ExternalOutput")

    RELU = mybir.ActivationFunctionType.Relu
    SIGM = mybir.ActivationFunctionType.Sigmoid

    with tile.TileContext(nc) as tc:
        with (
            tc.tile_pool(name="wts", bufs=1) as wts,
            tc.tile_pool(name="acts", bufs=1) as acts,
            tc.tile_pool(name="edge", bufs=6) as edge,
            tc.tile_pool(name="psA", bufs=2, space=bass.MemorySpace.PSUM) as psA,
            tc.tile_pool(name="psE", bufs=4, space=bass.MemorySpace.PSUM) as psE,
        ):
            # ---- loads: 5 packed DMAs across the two HWDGE rings -------
            bigA_t = wts.tile([128, 1280], F32, name="bigA", tag="bigA")
            nc.sync.dma_start(bigA_t[:], bigA.ap())
            bigB_t = wts.tile([128, 2052], BF16, name="bigB", tag="bigB")
            nc.scalar.dma_start(bigB_t[:], bigB.ap())
            bigC_t = wts.tile([128, 76], F32, name="bigC", tag="bigC")
            nc.scalar.dma_start(bigC_t[:], bigC.ap())
            cr_t = wts.tile([1, 160], F32, name="cr", tag="cr")
            nc.sync.dma_start(cr_t[:], cr.ap())
            tril_t = wts.tile([2, NN * NN], BF16, name="tril", tag="tril")
            nc.scalar.dma_start(tril_t[:], trilf.ap())
            zt_t = [bigA_t[:, 0:128], bigA_t[:, 640:768]]
            wz_t = [bigA_t[:, 128:640], bigA_t[:, 768:1280]]
            we1_t = [bigB_t[:, k * 512 : (k + 1) * 512] for k in range(4)]
            cbb_t = bigB_t[:, 2048:2052]
            cb_t = bigC_t[:, 0:12]
            wn_t = [bigC_t[:, 12 + k * 16 : 12 + (k + 1) * 16] for k in range(4)]

            # ---- hT[m] = relu(W_z[:, m].T @ zT + b_z[m])  fp32 ---------
            ht = [acts.tile([128, BS], F32, name=f"ht{m}", tag=f"ht{m}") for m in range(4)]
            htb = [acts.tile([128, BS], BF16, name=f"htb{m}", tag=f"htb{m}") for m in range(4)]
            for m in range(4):
                ps = psA.tile([128, BS], F32)
                for k in range(2):
                    nc.tensor.matmul(
                        ps[:],
                        wz_t[k][:, m * 128 : (m + 1) * 128],
                        zt_t[k],
                        start=(k == 0),
                        stop=(k == 1),
                    )
                nc.scalar.activation(ht[m][:], ps[:], RELU, bias=cb_t[:, m : m + 1])
                nc.vector.tensor_copy(htb[m][:], ht[m][:])

            # ---- e1T[m] = relu(W_e1f[:, m].T @ hT + b_e1[m])  bf16 -----
            e1 = [acts.tile([128, BS], BF16, name=f"e1{m}", tag=f"e1{m}") for m in range(4)]
            for m in range(4):
                ps = psA.tile([128, BS], F32)
                for k in range(4):
                    nc.tensor.matmul(
                        ps[:],
                        we1_t[k][:, m * 128 : (m + 1) * 128],
                        htb[k][:],
                        start=(k == 0),
                        stop=(k == 3),
                    )
                nc.scalar.activation(
                    e1[m][:], ps[:], RELU, bias=cb_t[:, 4 + m : 5 + m]
                )

            # ---- pT = sigmoid(W_e2.T @ e1T + b_e2)  [1, BS] ------------
            psp = psA.tile([1, BS], F32, bufs=1)
            for k in range(4):
                nc.tensor.matmul(
                    psp[:],
                    cbb_t[:, k : k + 1],
                    e1[k][:],
                    start=(k == 0),
                    stop=(k == 3),
                )
            pt = acts.tile([1, BS], F32, name="pt", tag="pt")
            nc.scalar.activation(pt[:], psp[:], SIGM, bias=cr_t[0:1, 144:145])
            # exact fp32 -> bf16 hi/lo split: p == hi + lo to ~2^-17 rel.
            # Compute engines cannot address partition 1 directly, so build
            # hi/lo on partition-0 tiles and assemble [2, BS] via two tiny
            # SBUF->SBUF DMAs on separate rings.
            pthi_b = acts.tile([1, BS], BF16, name="pthi_b", tag="pthi_b")
            nc.vector.tensor_copy(pthi_b[:], pt[:])
            pthi_f = acts.tile([1, BS], F32, name="pthi_f", tag="pthi_f")
            nc.vector.tensor_copy(pthi_f[:], pthi_b[:])
            plo_f = acts.tile([1, BS], F32, name="plo_f", tag="plo_f")
            nc.vector.tensor_sub(plo_f[:], pt[:], pthi_f[:])
            plo_b = acts.tile([1, BS], BF16, name="plo_b", tag="plo_b")
            nc.vector.tensor_copy(plo_b[:], plo_f[:])
            pt2 = acts.tile([2, BS], BF16, name="pt2", tag="pt2")
            nc.sync.dma_start(pt2[0:1, :], pthi_b[:])
            nc.scalar.dma_start(pt2[1:2, :], plo_b[:])

            # ---- node logits: nl = hT.T @ W_node (+ ones x b_node) fp32
            psn = psA.tile([128, T], F32, bufs=1)
            for k in range(4):
                nc.tensor.matmul(
                    psn[:], ht[k][:], wn_t[k], start=(k == 0), stop=False
                )
            nc.tensor.matmul(
                psn[:], cr_t[0:1, 16:144], cr_t[0:1, 0:16], start=False, stop=True
            )
            nt = acts.tile([128, NN * T], F32, name="nt", tag="nt")
            nc.vector.tensor_copy(nt[:, 0:T], psn[:])
            w = T
            while w < NN * T:
                nc.vector.tensor_copy(nt[:, w : 2 * w], nt[:, 0:w])
                w *= 2
            nc.gpsimd.dma_start(node_out.ap(), nt[:])

            # ---- edge: out[b, f] = (p_hi + p_lo)[b] * tril_flat[f] -----
            # 8 macro-chunks of 2048; each = 4 K=2 bf16 matmuls
            # [2,128]x[2,512] -> PSUM f32 -> copy (ACT/DVE) -> 1MB DMA.
            for c in range(8):
                et = edge.tile([128, 2048], F32, name="et", tag="et")
                for j in range(4):
                    f0 = c * 2048 + j * 512
                    pse = psE.tile([128, 512], F32)
                    nc.tensor.matmul(
                        pse[:],
                        pt2[:],
                        tril_t[:, f0 : f0 + 512],
                        start=True,
                        stop=True,
                    )
                    dst = et[:, j * 512 : (j + 1) * 512]
                    if (c * 4 + j) % 2 == 0:
                        nc.scalar.copy(dst, pse[:])
                    else:
                        nc.vector.tensor_copy(dst, pse[:])
                if c % 2 == 0:
                    nc.sync.dma_start(
                        edge_out.ap()[:, c * 2048 : (c + 1) * 2048], et[:]
                    )
                else:
                    nc.scalar.dma_start(
                        edge_out.ap()[:, c * 2048 : (c + 1) * 2048], et[:]
                    )

    _split_multi_waits(nc)
    return nc


def kernel(z, num_nodes, W_z, b_z, W_node, b_node, W_e1, b_e1, W_e2, b_e2):
    import ml_dtypes

    z = np.asarray(z, dtype=np.float32)
    W_z = np.asarray(W_z, dtype=np.float32)
    b_z = np.asarray(b_z, dtype=np.float32)
    W_node = np.asarray(W_node, dtype=np.float32)
    b_node = np.asarray(b_node, dtype=np.float32)
    W_e1 = np.asarray(W_e1, dtype=np.float32)
    b_e1 = np.asarray(b_e1, dtype=np.float32)
    W_e2 = np.asarray(W_e2, dtype=np.float32)
    b_e2 = np.asarray(b_e2, dtype=np.float32)
    n_nodes = int(num_nodes)
    assert n_nodes == NN, f"kernel hardcodes num_nodes={NN}, got {n_nodes}"
    assert z.shape == (B, L)

    # Host-side constant packing (tiny tensors, replicated to all cores).
    we1f = (W_e1[:H] + W_e1[H:]).astype(ml_dtypes.bfloat16)  # [H, H] bf16
    bigB = np.zeros((128, 2052), dtype=ml_dtypes.bfloat16)
    for k in range(4):
        bigB[:, k * 512 : (k + 1) * 512] = we1f[k * 128 : (k + 1) * 128, :]
    bigB[:, 2048:2052] = W_e2[:, 0].reshape(4, 128).T.astype(ml_dtypes.bfloat16)
    bigC = np.zeros((128, 76), dtype=np.float32)
    bigC[:, 0:4] = b_z.reshape(4, 128).T
    bigC[:, 4:8] = b_e1.reshape(4, 128).T
    bigC[:, 8:12] = W_e2[:, 0].reshape(4, 128).T
    for k in range(4):
        bigC[:, 12 + k * 16 : 12 + (k + 1) * 16] = W_node[k * 128 : (k + 1) * 128, :]
    cr = np.zeros((1, 160), dtype=np.float32)
    cr[0, 0:T] = b_node
    cr[0, 16:144] = 1.0
    cr[0, 144] = b_e2[0]
    mask = np.tril(np.ones((NN, NN), dtype=np.float32), -1).reshape(1, NN * NN)
    trilf = np.ascontiguousarray(np.tile(mask, (2, 1)).astype(ml_dtypes.bfloat16))

    if "nc" not in _NC_CACHE:
        _NC_CACHE["nc"] = _build_bass()
    nc = _NC_CACHE["nc"]

    shared = {
        "bigB": np.ascontiguousarray(bigB),
        "bigC": bigC,
        "cr": cr,
        "trilf": trilf,
    }
    in_maps = []
    for i in range(N_CORES):
        m = dict(shared)
        zt_i = z[i * BS : (i + 1) * BS].T  # [L, BS]
        bigA = np.zeros((128, 1280), dtype=np.float32)
        bigA[:, 0:128] = zt_i[0:128]
        bigA[:, 128:640] = W_z[0:128]
        bigA[:, 640:768] = zt_i[128:256]
        bigA[:, 768:1280] = W_z[128:256]
        m["bigA"] = bigA
        in_maps.append(m)

    res = run_bass_kernel_spmd(nc, in_maps, core_ids=list(range(N_CORES)), **_RUN_KWARGS)
    _NC_CACHE["last_result"] = res

    node_logits = np.concatenate(
        [res.results[i]["node_out"].reshape(BS, NN, T) for i in range(N_CORES)],
        axis=0,
    )
    edge_probs = np.concatenate(
        [res.results[i]["edge_out"].reshape(BS, NN, NN) for i in range(N_CORES)],
        axis=0,
    )
    return node_logits, edge_probs


# revision 18
# speedup vs baseline: 1.2189x; 1.1463x over previous
"""Trainium2 Bass kernel for nn_DAGDecoder (dense_mlp).

Reference computation (B=1024, L=256, H=512, N=128 nodes, T=16 types):
    h  = relu(z @ W_z + b_z)                     [B, H]
    node_logits[b, n, :] = h[b] @ W_node + b_node      (same for all n)
    p  = sigmoid(relu(cat(h, h) @ W_e1 + b_e1) @ W_e2 + b_e2)   [B, 1]
    edge_probs[b] = p[b] * tril(ones(N, N), -1)  [B, N, N]

Host-side algebraic folds:
    cat(h, h) @ W_e1 == h @ (W_e1[:H] + W_e1[H:])  -> one [H, H] matmul
    node_logits is a broadcast of a [B, T] matrix over the node axis.
    edge_probs is a rank-1 outer product p x tril_flat per batch row.

Sharding: pure data parallel over batch; 8 cores x 128 batch rows; the
batch shard exactly fills the 128 SBUF partitions. Weights replicated.

On-device layout is fully transposed (hT [H, Bs], e1T [H, Bs], pT [1, Bs])
so every matmul contraction dim sits on partitions. Precision plan:
  - h and node_logits path: fp32 matmuls (node output ~1e-7 rel)
  - e1/p path: bf16 matmuls with fp32 PSUM accumulate (feeds only the
    scalar p through a sigmoid; ~5e-4 rel)
  - edge outer product: near-exact via bf16 hi/lo split of p
    (p == p_hi + p_lo, each bf16; the tril mask is exact in bf16), K=2
    bf16 matmuls instead of the 4-pass fp32 LOW_HIGH path.
Biases ride either the scalar-engine activation (per-partition bias) or a
K=1 matmul accumulation (ones-row x bias-row) when the bias is along the
free axis.
"""

import numpy as np

import concourse.bass as bass
import concourse.mybir as mybir
import concourse.tile as tile
from concourse.bass_utils import run_bass_kernel_spmd
from concourse.vector_clock import ScopedClock

# ---------------------------------------------------------------------------
# Workaround: the walrus build in this container encodes at most ONE sync
# wait command per instruction ("Too many sync wait commands"). Tile emits
# multi-wait instructions routinely. Engines execute their stream in order,
# so hoisting extra waits onto same-engine NOPs immediately before the
# instruction is semantically identical.
_MAX_DRAIN_WAITS = 1


def _drain_and_barrier_split(self, tick_clock, wait_clock):
    drain_inst = self.nc.sync.drain()
    wait_clock.add_sem_waits(
        drain_inst.ins, ScopedClock({None: tick_clock.global_clock})
    )
    waits = list(drain_inst.ins.sync_info.on_wait)
    if len(waits) > _MAX_DRAIN_WAITS:
        drain_inst.ins.sync_info = mybir.SyncInfo(
            on_wait=waits[:_MAX_DRAIN_WAITS], on_update=[]
        )
        for i in range(_MAX_DRAIN_WAITS, len(waits), _MAX_DRAIN_WAITS):
            d2 = self.nc.sync.drain()
            d2.ins.sync_info = mybir.SyncInfo(
                on_wait=waits[i : i + _MAX_DRAIN_WAITS], on_update=[]
            )

    self.nc.all_engine_barrier()
    assert self.sems is not None
    popped = self.nc._tile_sem_poison_stack.pop()
    assert popped is self._sem_poison
    self.nc.clear_and_free_semaphores(list(self.sems.allocated().values()))


tile.TileContext._drain_and_barrier = _drain_and_barrier_split


def _split_multi_waits(nc):
    for f in nc.m.functions:
        for bb in f.blocks:
            old = list(bb.instructions)
            new = []
            changed = False
            for ins in old:
                si = ins.sync_info
                waits = list(si.on_wait) if si is not None else []
                if len(waits) > 1:
                    changed = True
                    for i, w in enumerate(waits[:-1]):
                        new.append(
                            mybir.InstNoOp(
                                name=f"{ins.name}_wsplit{i}",
                                engine=ins.engine,
                                bass_nofuse=True,
                                sync_info=mybir.SyncInfo(on_wait=[w], on_update=[]),
                            )
                        )
                    ins.sync_info = mybir.SyncInfo(
                        on_wait=[waits[-1]], on_update=list(si.on_update)
                    )
                new.append(ins)
            if changed:
                bb.instructions = new


# ---------------------------------------------------------------------------

N_CORES = 8
B = 1024
BS = B // N_CORES  # 128 batch rows per core == SBUF partition count
L = 256  # latent
H = 512  # hidden
T = 16  # node types
NN = 128  # num_nodes

F32 = mybir.dt.float32
BF16 = mybir.dt.bfloat16
_NC_CACHE = {}
# Extra kwargs for run_bass_kernel_spmd (test harness sets trace=True here).
_RUN_KWARGS = {}


def _build_bass():
    nc = bass.Bass("TRN2", target_bir_lowering=False, debug=False)

    # bigA: [zt0 | wz0 | zt1 | wz1] columns (f32); bigB: [we1 k-tiles | cbb]
    # (bf16); bigC: [cb | wn k-tiles] (f32). Packing makes each load ONE
    # large DMA instead of many sequencer-serialized issues.
    bigA = nc.dram_tensor("bigA", [128, 1280], F32, kind="ExternalInput")
    bigB = nc.dram_tensor("bigB", [128, 2056], BF16, kind="ExternalInput")
    bigC = nc.dram_tensor("bigC", [128, 76], F32, kind="ExternalInput")
    cr = nc.dram_tensor("cr", [1, 160], F32, kind="ExternalInput")
    trilf = nc.dram_tensor("trilf", [2, NN * NN], BF16, kind="ExternalInput")

    node_out = nc.dram_tensor("node_out", [BS, NN * T], F32, kind="ExternalOutput")
    edge_out = nc.dram_tensor("edge_out", [BS, NN * NN], F32, kind="ExternalOutput")

    RELU = mybir.ActivationFunctionType.Relu
    SIGM = mybir.ActivationFunctionType.Sigmoid

    with tile.TileContext(nc) as tc:
        with (
            tc.tile_pool(name="wts", bufs=1) as wts,
            tc.tile_pool(name="acts", bufs=1) as acts,
            tc.tile_pool(name="edge", bufs=6) as edge,
            tc.tile_pool(name="psA", bufs=2, space=bass.MemorySpace.PSUM) as psA,
            tc.tile_pool(name="psE", bufs=4, space=bass.MemorySpace.PSUM) as psE,
        ):
            # ---- loads: column-blocked so the first h-matmul's operands
            # land first; spread across both HWDGE rings ------------------
            bigA_t = wts.tile([128, 1280], F32, name="bigA", tag="bigA")
            nc.sync.dma_start(bigA_t[:, 0:512], bigA.ap()[:, 0:512])
            bigB_t = wts.tile([128, 2056], BF16, name="bigB", tag="bigB")
            nc.scalar.dma_start(bigB_t[:], bigB.ap())
            nc.sync.dma_start(bigA_t[:, 512:896], bigA.ap()[:, 512:896])
            nc.sync.dma_start(bigA_t[:, 896:1280], bigA.ap()[:, 896:1280])
            bigC_t = wts.tile([128, 76], F32, name="bigC", tag="bigC")
            nc.scalar.dma_start(bigC_t[:], bigC.ap())
            cr_t = wts.tile([1, 160], F32, name="cr", tag="cr")
            nc.sync.dma_start(cr_t[:], cr.ap())
            tril_t = wts.tile([2, NN * NN], BF16, name="tril", tag="tril")
            nc.scalar.dma_start(tril_t[:], trilf.ap())
            zt_t = [bigA_t[:, 0:128], bigA_t[:, 128:256]]
            # wz column-block (m, k): bigA[:, 256 + (m*2+k)*128 : +128]
            wz_mk = lambda m, k: bigA_t[:, 256 + (m * 2 + k) * 128 : 256 + (m * 2 + k + 1) * 128]
            we1_t = [bigB_t[:, k * 512 : (k + 1) * 512] for k in range(4)]
            cbb_t = bigB_t[:, 2048:2052]
            ehi_t = bigB_t[0:1, 2052:2054]
            elo_t = bigB_t[0:1, 2054:2056]
            cb_t = bigC_t[:, 0:12]
            wn_t = [bigC_t[:, 12 + k * 16 : 12 + (k + 1) * 16] for k in range(4)]

            # ---- hT[m] = relu(W_z[:, m].T @ zT + b_z[m])  fp32 ---------
            ht = [acts.tile([128, BS], F32, name=f"ht{m}", tag=f"ht{m}") for m in range(4)]
            htb = [acts.tile([128, BS], BF16, name=f"htb{m}", tag=f"htb{m}") for m in range(4)]
            for m in range(4):
                ps = psA.tile([128, BS], F32)
                for k in range(2):
                    nc.tensor.matmul(
                        ps[:],
                        wz_mk(m, k),
                        zt_t[k],
                        start=(k == 0),
                        stop=(k == 1),
                    )
                nc.scalar.activation(ht[m][:], ps[:], RELU, bias=cb_t[:, m : m + 1])
                nc.vector.tensor_copy(htb[m][:], ht[m][:])

            # ---- e1T[m] = relu(W_e1f[:, m].T @ hT + b_e1[m])  bf16 -----
            e1 = [acts.tile([128, BS], BF16, name=f"e1{m}", tag=f"e1{m}") for m in range(4)]
            for m in range(4):
                ps = psA.tile([128, BS], F32)
                for k in range(4):
                    nc.tensor.matmul(
                        ps[:],
                        we1_t[k][:, m * 128 : (m + 1) * 128],
                        htb[k][:],
                        start=(k == 0),
                        stop=(k == 3),
                    )
                nc.scalar.activation(
                    e1[m][:], ps[:], RELU, bias=cb_t[:, 4 + m : 5 + m]
                )

            # ---- pT = sigmoid(W_e2.T @ e1T + b_e2)  [1, BS] ------------
            psp = psA.tile([1, BS], F32, bufs=1)
            for k in range(4):
                nc.tensor.matmul(
                    psp[:],
                    cbb_t[:, k : k + 1],
                    e1[k][:],
                    start=(k == 0),
                    stop=(k == 3),
                )
            pt = acts.tile([1, BS], F32, name="pt", tag="pt")
            nc.scalar.activation(pt[:], psp[:], SIGM, bias=cr_t[0:1, 144:145])
            # exact fp32 -> bf16 hi/lo split: p == hi + lo to ~2^-17 rel.
            # Compute engines cannot address partition 1 directly, so build
            # hi/lo on partition-0 tiles and assemble [2, BS] via two tiny
            # SBUF->SBUF DMAs on separate rings.
            pthi_b = acts.tile([1, BS], BF16, name="pthi_b", tag="pthi_b")
            nc.vector.tensor_copy(pthi_b[:], pt[:])
            pthi_f = acts.tile([1, BS], F32, name="pthi_f", tag="pthi_f")
            nc.vector.tensor_copy(pthi_f[:], pthi_b[:])
            plo_f = acts.tile([1, BS], F32, name="plo_f", tag="plo_f")
            nc.vector.tensor_sub(plo_f[:], pt[:], pthi_f[:])
            plo_b = acts.tile([1, BS], BF16, name="plo_b", tag="plo_b")
            nc.vector.tensor_copy(plo_b[:], plo_f[:])
            psq = psA.tile([2, BS], F32, bufs=1, name="psq", tag="psp")
            nc.tensor.matmul(psq[:], ehi_t, pthi_b[:], start=True, stop=False)
            nc.tensor.matmul(psq[:], elo_t, plo_b[:], start=False, stop=True)
            pt2 = acts.tile([2, BS], BF16, name="pt2", tag="pt2")
            nc.vector.tensor_copy(pt2[:], psq[:])

            # ---- edge: out[b, f] = (p_hi + p_lo)[b] * tril_flat[f] -----
            # 8 macro-chunks of 2048; each = 4 K=2 bf16 matmuls
            # [2,128]x[2,512] -> PSUM f32 -> copy (ACT/DVE) -> 1MB DMA.
            for c in range(8):
                et = edge.tile([128, 2048], F32, name="et", tag="et")
                for j in range(4):
                    f0 = c * 2048 + j * 512
                    pse = psE.tile([128, 512], F32)
                    nc.tensor.matmul(
                        pse[:],
                        pt2[:],
                        tril_t[:, f0 : f0 + 512],
                        start=True,
                        stop=True,
                    )
                    dst = et[:, j * 512 : (j + 1) * 512]
                    if (c * 4 + j) % 2 == 0:
                        nc.scalar.copy(dst, pse[:])
                    else:
                        nc.vector.tensor_copy(dst, pse[:])
                if c % 2 == 0:
                    nc.sync.dma_start(
                        edge_out.ap()[:, c * 2048 : (c + 1) * 2048], et[:]
                    )
                else:
                    nc.scalar.dma_start(
                        edge_out.ap()[:, c * 2048 : (c + 1) * 2048], et[:]
                    )

            # ---- node logits (after edge: keeps DVE free for the p chain): nl = hT.T @ W_node (+ ones x b_node) fp32
            psn = psA.tile([128, T], F32, bufs=1)
            for k in range(4):
                nc.tensor.matmul(
                    psn[:], ht[k][:], wn_t[k], start=(k == 0), stop=False
                )
            nc.tensor.matmul(
                psn[:], cr_t[0:1, 16:144], cr_t[0:1, 0:16], start=False, stop=True
            )
            nt = acts.tile([128, NN * T], F32, name="nt", tag="nt")
            nc.vector.tensor_copy(nt[:, 0:T], psn[:])
            w = T
            while w < NN * T:
                nc.vector.tensor_copy(nt[:, w : 2 * w], nt[:, 0:w])
                w *= 2
            nc.gpsimd.dma_start(node_out.ap(), nt[:])


    _split_multi_waits(nc)
    return nc


def kernel(z, num_nodes, W_z, b_z, W_node, b_node, W_e1, b_e1, W_e2, b_e2):
    import ml_dtypes

    z = np.asarray(z, dtype=np.float32)
    W_z = np.asarray(W_z, dtype=np.float32)
    b_z = np.asarray(b_z, dtype=np.float32)
    W_node = np.asarray(W_node, dtype=np.float32)
    b_node = np.asarray(b_node, dtype=np.float32)
    W_e1 = np.asarray(W_e1, dtype=np.float32)
    b_e1 = np.asarray(b_e1, dtype=np.float32)
    W_e2 = np.asarray(W_e2, dtype=np.float32)
    b_e2 = np.asarray(b_e2, dtype=np.float32)
    n_nodes = int(num_nodes)
    assert n_nodes == NN, f"kernel hardcodes num_nodes={NN}, got {n_nodes}"
    assert z.shape == (B, L)

    # Host-side constant packing (tiny tensors, replicated to all cores).
    we1f = (W_e1[:H] + W_e1[H:]).astype(ml_dtypes.bfloat16)  # [H, H] bf16
    bigB = np.zeros((128, 2056), dtype=ml_dtypes.bfloat16)
    for k in range(4):
        bigB[:, k * 512 : (k + 1) * 512] = we1f[k * 128 : (k + 1) * 128, :]
    bigB[:, 2048:2052] = W_e2[:, 0].reshape(4, 128).T.astype(ml_dtypes.bfloat16)
    bigB[0, 2052] = 1.0  # e_hi = [1, 0]
    bigB[0, 2055] = 1.0  # e_lo = [0, 1]
    bigC = np.zeros((128, 76), dtype=np.float32)
    bigC[:, 0:4] = b_z.reshape(4, 128).T
    bigC[:, 4:8] = b_e1.reshape(4, 128).T
    bigC[:, 8:12] = W_e2[:, 0].reshape(4, 128).T
    for k in range(4):
        bigC[:, 12 + k * 16 : 12 + (k + 1) * 16] = W_node[k * 128 : (k + 1) * 128, :]
    cr = np.zeros((1, 160), dtype=np.float32)
    cr[0, 0:T] = b_node
    cr[0, 16:144] = 1.0
    cr[0, 144] = b_e2[0]
    mask = np.tril(np.ones((NN, NN), dtype=np.float32), -1).reshape(1, NN * NN)
    trilf = np.ascontiguousarray(np.tile(mask, (2, 1)).astype(ml_dtypes.bfloat16))

    if "nc" not in _NC_CACHE:
        _NC_CACHE["nc"] = _build_bass()
    nc = _NC_CACHE["nc"]

    shared = {
        "bigB": np.ascontiguousarray(bigB),
        "bigC": bigC,
        "cr": cr,
        "trilf": trilf,
    }
    in_maps = []
    for i in range(N_CORES):
        m = dict(shared)
        zt_i = z[i * BS : (i + 1) * BS].T  # [L, BS]
        bigA = np.zeros((128, 1280), dtype=np.float32)
        bigA[:, 0:128] = zt_i[0:128]
        bigA[:, 128:256] = zt_i[128:256]
        for mm in range(4):
            for kk in range(2):
                c0 = 256 + (mm * 2 + kk) * 128
                bigA[:, c0 : c0 + 128] = W_z[
                    kk * 128 : (kk + 1) * 128, mm * 128 : (mm + 1) * 128
                ]
        m["bigA"] = bigA
        in_maps.append(m)

    res = run_bass_kernel_spmd(nc, in_maps, core_ids=list(range(N_CORES)), **_RUN_KWARGS)
    _NC_CACHE["last_result"] = res

    node_logits = np.concatenate(
        [res.results[i]["node_out"].reshape(BS, NN, T) for i in range(N_CORES)],
        axis=0,
    )
    edge_probs = np.concatenate(
        [res.results[i]["edge_out"].reshape(BS, NN, NN) for i in range(N_CORES)],
        axis=0,
    )
    return node_logits, edge_probs
